# revision 14
# baseline (speedup 1.0000x reference)
"""QRNN forget-mult kernel for Trainium2 (Bass/Tile), 8-core batch-parallel.

Reference computation (per batch b):
    x = tanh(inputs @ W_in.T + b_in)            # (T, D)
    f = sigmoid(inputs @ W_f.T + b_f + 10000*mask)
    h_t = f_t*x_t + (1-f_t)*h_{t-1},  h_{-1} = 0

Shapes: B=8, T=4096, D_IN=D_OUT=256, fp32.

Sharding: batch across the 8 NeuronCores (core c <- batch c). The
recurrence is independent per (batch, feature) so no communication.

Per-core dataflow, with engine work balanced so everything sits at or
under the ~24.5us DMA roofline (4MB in + 4MB out + weights @ 360GB/s):
  DMA in    : inputs[c] natural [128t, d] chunks            (SP HWDGE)
  PE        : transpose input tiles -> psT [128d, t] (fp32r, 1.5cyc/row)
  DVE/ACT   : copy psT PSUM->SBUF rhs (split across chunks to balance)
  PE        : z_x, z_f = W^T.T @ rhs accumulated over d (fp32r 1cyc/row)
  ACT       : xg = tanh(z_x + b_in); fg = sigmoid(z_f + b_f)  -> bf16
  DVE       : ag = 1 - fg          (tensor_scalar, 4x bf16 mode)
  Pool      : bn = fg * xg         (tensor_tensor)
  DVE       : H = scan(ag, bn): H_t = ag_t*H_{t-1} + bn_t (fp32 state,
              bf16 stored output)
  PE        : transpose H (bf16 -> 1 cyc/row) -> PSUM bf16
  DVE       : copy PSUM->SBUF bf16 (2x mode)
  Pool SWDGE: casting DMA bf16 -> fp32 natural [t, o] rows (exact widen)
"""

import os
import sys

import numpy as np

for _p in ("/opt/trn_rl_repo",):
    if _p not in sys.path and os.path.isdir(_p):
        sys.path.insert(0, _p)

import concourse.bacc as bacc
import concourse.bass as bass
import concourse.mybir as mybir
import concourse.tile as tile
from concourse.bass_utils import run_bass_kernel_spmd
from concourse.masks import make_identity

B, T, D = 8, 4096, 256
N_CORES = 8
TC = 512          # time-chunk per pipeline iteration
N_CHUNKS = T // TC
F32 = mybir.dt.float32
F32R = mybir.dt.float32r
BF16 = mybir.dt.bfloat16

# chunks whose psT->rhs copy runs on DVE (rest on ACT); tuned for balance
W1_DVE = {0, 3, 6}

_cache = {}


def build_module(with_mask: bool):
    nc = bacc.Bacc("TRN2")

    # x and the weight matrices are declared float32r (same 4-byte layout,
    # np.float32 on the host): their transposes then run in the faster
    # 1.5 cyc/row fp32r PE mode and satisfy the fp32r producer-rounding rule
    x_in = nc.dram_tensor("x", [T, D], F32R, kind="ExternalInput")
    w_in = nc.dram_tensor("w_in", [D, D], F32R, kind="ExternalInput")
    b_in = nc.dram_tensor("b_in", [D], F32, kind="ExternalInput")
    w_f = nc.dram_tensor("w_f", [D, D], F32R, kind="ExternalInput")
    b_f = nc.dram_tensor("b_f", [D], F32, kind="ExternalInput")
    mask = None
    if with_mask:
        mask = nc.dram_tensor("mask", [T, 1], F32, kind="ExternalInput")
    out = nc.dram_tensor("out", [T, D], F32, kind="ExternalOutput")

    with tile.TileContext(nc) as tc:
        with (
            tc.tile_pool(name="consts", bufs=1) as consts,
            tc.tile_pool(name="persist", bufs=1) as persist,
            tc.tile_pool(name="nat", bufs=6) as nat_pool,
            tc.tile_pool(name="rhs", bufs=3) as rhs_pool,
            tc.tile_pool(name="gates", bufs=3) as gate_pool,
            tc.tile_pool(name="onat", bufs=3) as onat_pool,
            tc.tile_pool(name="ps_in", bufs=2, space="PSUM") as ps_in,
            tc.tile_pool(name="ps_z", bufs=2, space="PSUM") as ps_z,
            tc.tile_pool(name="ps_out", bufs=2, space="PSUM") as ps_out,
        ):
            # ---- one-time setup -------------------------------------
            def cst(shape, dtype, nm):
                return consts.tile(shape, dtype, name=nm, tag=nm)

            # identity for fp32 transposes, an fp32r-rounded copy for the
            # fp32r input transposes (verifier: fp32r matmul operands must
            # come from an fp32r-rounding producer), and a bf16 copy for
            # the 1 cyc/row output transposes
            ident = cst([128, 128], F32, "ident")
            make_identity(nc, ident)
            ident_r = cst([128, 128], F32R, "ident_r")
            nc.vector.tensor_copy(ident_r, ident)
            ident_h = cst([128, 128], BF16, "ident_h")
            nc.vector.tensor_copy(ident_h, ident)

            # negated biases [128, 1] per o-half: the activations compute
            # ag = sigmoid(-z_f - b_f) = 1 - f and xn = tanh(-z_x - b_in)
            # = -x via scale=-1, so the gate math needs -b
            bias_x = []
            bias_f = []
            for oh in range(2):
                bx = cst([128, 1], F32, f"bx{oh}")
                nc.gpsimd.dma_start(
                    out=bx, in_=bass.AP(b_in, oh * 128, [[1, 128], [0, 1]])
                )
                bxn = cst([128, 1], F32, f"bxn{oh}")
                nc.vector.tensor_scalar(
                    bxn, bx, -1.0, 0.0,
                    op0=mybir.AluOpType.mult, op1=mybir.AluOpType.add,
                )
                bf = cst([128, 1], F32, f"bf{oh}")
                nc.gpsimd.dma_start(
                    out=bf, in_=bass.AP(b_f, oh * 128, [[1, 128], [0, 1]])
                )
                bfn = cst([128, 1], F32, f"bfn{oh}")
                nc.vector.tensor_scalar(
                    bfn, bf, -1.0, 0.0,
                    op0=mybir.AluOpType.mult, op1=mybir.AluOpType.add,
                )
                bias_x.append(bxn)
                bias_f.append(bfn)

            # weights: load natural [128o, 256d], PE-transpose to
            # wT[gate][kh] = [128d, 256o]
            wT = [[None, None], [None, None]]
            for g, w_dram in enumerate((w_in, w_f)):
                wnat = []
                for oh in range(2):
                    wn = cst([128, D], F32R, f"wnat{g}{oh}")
                    nc.gpsimd.dma_start(
                        out=wn, in_=w_dram[oh * 128 : (oh + 1) * 128, :]
                    )
                    wnat.append(wn)
                for kh in range(2):
                    pw = ps_in.tile([128, 2 * TC], F32R, tag="psT", name=f"pw{g}{kh}")
                    for oh in range(2):
                        nc.tensor.transpose(
                            pw[:, oh * 128 : (oh + 1) * 128],
                            wnat[oh][:, kh * 128 : (kh + 1) * 128],
                            ident_r,
                        )
                    wt = cst([128, D], F32R, f"wT{g}{kh}")
                    nc.vector.tensor_copy(wt, pw[:, 0:D])
                    wT[g][kh] = wt

            mask_sb = None
            ones10k = None
            if with_mask:
                mask_sb = persist.tile([1, T], F32R, tag="mask_sb", name="mask_sb")
                nc.gpsimd.dma_start(
                    out=mask_sb, in_=bass.AP(mask, 0, [[0, 1], [1, T]])
                )
                ones10k = cst([1, 128], F32, "ones10k_f")
                nc.vector.memset(ones10k, 10000.0)
                ones10k_r = cst([1, 128], F32R, "ones10k")
                nc.vector.tensor_copy(ones10k_r, ones10k)
                ones10k = ones10k_r

            # pin the ACT function table: sigmoid_and_others contains BOTH
            # Sigmoid and Tanh, so forcing Sigmoid first avoids a second
            # 1.3us table load when Tanh would otherwise pick its own table
            actpin = cst([128, 1], F32, "actpin")
            nc.scalar.activation(
                actpin, bias_x[0], mybir.ActivationFunctionType.Sigmoid
            )

            # persistent scan output, per o-half strip; bf16 so the output
            # transposes run in the 1 cyc/row PE mode (the scan state is
            # fp32 internally; only the stored H is rounded)
            H = [
                persist.tile([128, T], BF16, tag=f"H{oh}", name=f"H{oh}")
                for oh in range(2)
            ]

            NB = TC // 128  # t-blocks per chunk
            x_v = x_in[:, :].rearrange("(c n p) d -> c p n d", p=128, n=NB)
            out_v = out[:, :].rearrange("(q n p) o -> q p n o", p=128, n=NB)


            # ---- main pipeline --------------------------------------
            for c in range(N_CHUNKS):
                t0 = c * TC
                nat = nat_pool.tile([128, NB, D], F32R, tag="nat", name=f"nat{c}")
                nc.sync.dma_start(out=nat, in_=x_v[c])

                # input transpose: [128t, 128d] blocks -> psT [128d, 2*TC]
                # (kh halves side by side), one PSUM->SBUF copy per chunk
                pt = ps_in.tile([128, 2 * TC], F32R, tag="psT")
                for kh in range(2):
                    for n in range(NB):
                        nc.tensor.transpose(
                            pt[:, kh * TC + n * 128 : kh * TC + (n + 1) * 128],
                            nat[:, n, kh * 128 : (kh + 1) * 128],
                            ident_r,
                        )
                rhs = rhs_pool.tile([128, 2 * TC], F32R, tag="rs", name=f"rs{c}")
                if c in W1_DVE:
                    nc.vector.tensor_copy(rhs, pt)
                else:
                    nc.scalar.copy(rhs, pt)

                # gates for both o-halves live side by side on the free
                # axis: [:, 0:TC] is oh=0, [:, TC:2TC] is oh=1.
                # xn = tanh(-z_x - b_in) = -x, ag = sigmoid(-z_f - b_f)
                # = 1 - f: both negations are free via the ACT scale input
                xn = gate_pool.tile([128, 2 * TC], BF16, tag="xn")
                ag = gate_pool.tile([128, 2 * TC], BF16, tag="ag")
                for oh in range(2):
                    sl = slice(oh * TC, (oh + 1) * TC)
                    # z_f -> 1-sigmoid (emitted first: the first activation
                    # executed being Sigmoid pins the sigmoid_and_others
                    # table, which also serves Tanh -> one table load)
                    zf = ps_z.tile([128, TC], F32, tag="z")
                    n_acc = 3 if with_mask else 2
                    for kh in range(2):
                        nc.tensor.matmul(
                            zf,
                            wT[1][kh][:, oh * 128 : (oh + 1) * 128],
                            rhs[:, kh * TC : (kh + 1) * TC],
                            start=(kh == 0),
                            stop=(kh == n_acc - 1),
                        )
                    if with_mask:
                        nc.tensor.matmul(
                            zf,
                            ones10k,
                            mask_sb[:, t0 : t0 + TC],
                            start=False,
                            stop=True,
                        )
                    nc.scalar.activation(
                        ag[:, sl], zf, mybir.ActivationFunctionType.Sigmoid,
                        bias=bias_f[oh], scale=-1.0,
                    )

                    # z_x -> -tanh
                    z = ps_z.tile([128, TC], F32, tag="z")
                    for kh in range(2):
                        nc.tensor.matmul(
                            z,
                            wT[0][kh][:, oh * 128 : (oh + 1) * 128],
                            rhs[:, kh * TC : (kh + 1) * TC],
                            start=(kh == 0),
                            stop=(kh == 1),
                        )
                    nc.scalar.activation(
                        xn[:, sl], z, mybir.ActivationFunctionType.Tanh,
                        bias=bias_x[oh], scale=-1.0,
                    )

                # b = f*x = (ag - 1) * (-x): tensor_scalar runs in 4x bf16
                # mode and tensor_tensor in 2x, beating the fused 1x op
                am = gate_pool.tile([128, 2 * TC], BF16, tag="am")
                nc.vector.tensor_scalar(
                    am, ag, 1.0, 0.0,
                    op0=mybir.AluOpType.subtract,
                    op1=mybir.AluOpType.add,
                )
                bn = gate_pool.tile([128, 2 * TC], BF16, tag="bn")
                nc.vector.tensor_tensor(bn, am, xn, op=mybir.AluOpType.mult)

                # h_t = a*h_{t-1} + b  (fp32 internal state per scan instr)
                for oh in range(2):
                    sl = slice(oh * TC, (oh + 1) * TC)
                    init = 0.0 if c == 0 else H[oh][:, t0 - 1 : t0]
                    nc.vector.tensor_tensor_scan(
                        H[oh][:, t0 : t0 + TC],
                        ag[:, sl],
                        bn[:, sl],
                        init,
                        op0=mybir.AluOpType.mult,
                        op1=mybir.AluOpType.add,
                    )

                # output transpose (bf16, 1 cyc/row): po[t, n*256+o] rows
                po = ps_out.tile([128, NB * 256], BF16)
                for n in range(NB):
                    tb = t0 + n * 128
                    for oh in range(2):
                        nc.tensor.transpose(
                            po[:, n * 256 + oh * 128 : n * 256 + oh * 128 + 128],
                            H[oh][:, tb : tb + 128],
                            ident_h,
                        )
                # stage bf16 PSUM -> SBUF (DVE 2x mode), then flush with a
                # casting SWDGE DMA (bf16 -> fp32, exact widening)
                onat = onat_pool.tile([128, NB, 256], BF16, tag="onat",
                                      name=f"onat{c}")
                nc.vector.tensor_copy(
                    onat.rearrange("p n o -> p (n o)"), po
                )
                nc.gpsimd.dma_start(out=out_v[c], in_=onat)

    nc.compile()
    return nc


def _get_module(with_mask: bool):
    key = bool(with_mask)
    if key not in _cache:
        _cache[key] = build_module(key)
    return _cache[key]


def kernel(**inputs):
    inp = np.ascontiguousarray(np.asarray(inputs["inputs"], dtype=np.float32))
    msk = np.ascontiguousarray(np.asarray(inputs["mask"], dtype=np.float32))
    w_in = np.ascontiguousarray(np.asarray(inputs["W_in"], dtype=np.float32))
    b_in = np.ascontiguousarray(np.asarray(inputs["b_in"], dtype=np.float32))
    w_f = np.ascontiguousarray(np.asarray(inputs["W_f"], dtype=np.float32))
    b_f = np.ascontiguousarray(np.asarray(inputs["b_f"], dtype=np.float32))

    with_mask = bool(np.any(msk != 0.0))
    nc = _get_module(with_mask)

    in_maps = []
    for c in range(N_CORES):
        m = {
            "x": inp[c],
            "w_in": w_in,
            "b_in": b_in,
            "w_f": w_f,
            "b_f": b_f,
        }
        if with_mask:
            m["mask"] = msk[c]
        in_maps.append(m)

    res = run_bass_kernel_spmd(nc, in_maps, core_ids=list(range(N_CORES)))
    return np.stack([res.results[c]["out"] for c in range(N_CORES)], axis=0)


# revision 15
# speedup vs baseline: 1.0535x; 1.0535x over previous
"""QRNN forget-mult kernel for Trainium2 (Bass/Tile), 8-core batch-parallel.

Reference computation (per batch b):
    x = tanh(inputs @ W_in.T + b_in)            # (T, D)
    f = sigmoid(inputs @ W_f.T + b_f + 10000*mask)
    h_t = f_t*x_t + (1-f_t)*h_{t-1},  h_{-1} = 0

Shapes: B=8, T=4096, D_IN=D_OUT=256, fp32.

Sharding: batch across the 8 NeuronCores (core c <- batch c). The
recurrence is independent per (batch, feature) so no communication.

Per-core dataflow, with engine work balanced so everything sits at or
under the ~24.5us DMA roofline (4MB in + 4MB out + weights @ 360GB/s):
  DMA in    : inputs[c] natural [128t, d] chunks            (SP HWDGE)
  PE        : transpose input tiles -> psT [128d, t] (fp32r, 1.5cyc/row)
  DVE/ACT   : copy psT PSUM->SBUF rhs (split across chunks to balance)
  PE        : z_x, z_f = W^T.T @ rhs accumulated over d (fp32r 1cyc/row)
  ACT       : xg = tanh(z_x + b_in); fg = sigmoid(z_f + b_f)  -> bf16
  DVE       : ag = 1 - fg          (tensor_scalar, 4x bf16 mode)
  Pool      : bn = fg * xg         (tensor_tensor)
  DVE       : H = scan(ag, bn): H_t = ag_t*H_{t-1} + bn_t (fp32 state,
              bf16 stored output)
  PE        : transpose H (bf16 -> 1 cyc/row) -> PSUM bf16
  DVE       : copy PSUM->SBUF bf16 (2x mode)
  Pool SWDGE: casting DMA bf16 -> fp32 natural [t, o] rows (exact widen)
"""

import os
import sys

import numpy as np

for _p in ("/opt/trn_rl_repo",):
    if _p not in sys.path and os.path.isdir(_p):
        sys.path.insert(0, _p)

import concourse.bacc as bacc
import concourse.bass as bass
import concourse.mybir as mybir
import concourse.tile as tile
from concourse.bass_utils import run_bass_kernel_spmd
from concourse.masks import make_identity

B, T, D = 8, 4096, 256
N_CORES = 8
TC = 512          # time-chunk per pipeline iteration
N_CHUNKS = T // TC
F32 = mybir.dt.float32
F32R = mybir.dt.float32r
BF16 = mybir.dt.bfloat16

# chunks whose psT->rhs copy runs on DVE (rest on ACT); tuned for balance
W1_DVE = {0, 3, 6}

_cache = {}


def build_module(with_mask: bool):
    nc = bacc.Bacc("TRN2")

    # x and the weight matrices are declared float32r (same 4-byte layout,
    # np.float32 on the host): their transposes then run in the faster
    # 1.5 cyc/row fp32r PE mode and satisfy the fp32r producer-rounding rule
    x_in = nc.dram_tensor("x", [T, D], F32R, kind="ExternalInput")
    w_in = nc.dram_tensor("w_in", [D, D], F32R, kind="ExternalInput")
    b_in = nc.dram_tensor("b_in", [D], F32, kind="ExternalInput")
    w_f = nc.dram_tensor("w_f", [D, D], F32R, kind="ExternalInput")
    b_f = nc.dram_tensor("b_f", [D], F32, kind="ExternalInput")
    mask = None
    if with_mask:
        mask = nc.dram_tensor("mask", [T, 1], F32, kind="ExternalInput")
    out = nc.dram_tensor("out", [T, D], F32, kind="ExternalOutput")

    with tile.TileContext(nc) as tc:
        with (
            tc.tile_pool(name="consts", bufs=1) as consts,
            tc.tile_pool(name="persist", bufs=1) as persist,
            tc.tile_pool(name="nat", bufs=6) as nat_pool,
            tc.tile_pool(name="rhs", bufs=3) as rhs_pool,
            tc.tile_pool(name="gates", bufs=3) as gate_pool,
            tc.tile_pool(name="onat", bufs=3) as onat_pool,
            tc.tile_pool(name="ps_in", bufs=2, space="PSUM") as ps_in,
            tc.tile_pool(name="ps_z", bufs=2, space="PSUM") as ps_z,
            tc.tile_pool(name="ps_out", bufs=2, space="PSUM") as ps_out,
        ):
            # ---- one-time setup -------------------------------------
            def cst(shape, dtype, nm):
                return consts.tile(shape, dtype, name=nm, tag=nm)

            # identity for fp32 transposes, an fp32r-rounded copy for the
            # fp32r input transposes (verifier: fp32r matmul operands must
            # come from an fp32r-rounding producer), and a bf16 copy for
            # the 1 cyc/row output transposes
            ident = cst([128, 128], F32, "ident")
            make_identity(nc, ident)
            ident_r = cst([128, 128], F32R, "ident_r")
            nc.vector.tensor_copy(ident_r, ident)
            ident_h = cst([128, 128], BF16, "ident_h")
            nc.vector.tensor_copy(ident_h, ident)

            # negated biases [128, 1] per o-half: the activations compute
            # ag = sigmoid(-z_f - b_f) = 1 - f and xn = tanh(-z_x - b_in)
            # = -x via scale=-1, so the gate math needs -b
            bias_x = []
            bias_f = []
            for oh in range(2):
                bx = cst([128, 1], F32, f"bx{oh}")
                nc.scalar.dma_start(
                    out=bx, in_=bass.AP(b_in, oh * 128, [[1, 128], [0, 1]])
                )
                bxn = cst([128, 1], F32, f"bxn{oh}")
                nc.vector.tensor_scalar(
                    bxn, bx, -1.0, 0.0,
                    op0=mybir.AluOpType.mult, op1=mybir.AluOpType.add,
                )
                bf = cst([128, 1], F32, f"bf{oh}")
                nc.scalar.dma_start(
                    out=bf, in_=bass.AP(b_f, oh * 128, [[1, 128], [0, 1]])
                )
                bfn = cst([128, 1], F32, f"bfn{oh}")
                nc.vector.tensor_scalar(
                    bfn, bf, -1.0, 0.0,
                    op0=mybir.AluOpType.mult, op1=mybir.AluOpType.add,
                )
                bias_x.append(bxn)
                bias_f.append(bfn)

            # weights: load natural [128o, 256d], PE-transpose to
            # wT[gate][kh] = [128d, 256o]
            wT = [[None, None], [None, None]]
            for g, w_dram in enumerate((w_in, w_f)):
                wnat = []
                for oh in range(2):
                    wn = cst([128, D], F32R, f"wnat{g}{oh}")
                    nc.scalar.dma_start(
                        out=wn, in_=w_dram[oh * 128 : (oh + 1) * 128, :]
                    )
                    wnat.append(wn)
                for kh in range(2):
                    pw = ps_in.tile([128, 2 * TC], F32R, tag="psT", name=f"pw{g}{kh}")
                    for oh in range(2):
                        nc.tensor.transpose(
                            pw[:, oh * 128 : (oh + 1) * 128],
                            wnat[oh][:, kh * 128 : (kh + 1) * 128],
                            ident_r,
                        )
                    wt = cst([128, D], F32R, f"wT{g}{kh}")
                    nc.vector.tensor_copy(wt, pw[:, 0:D])
                    wT[g][kh] = wt

            mask_sb = None
            ones10k = None
            if with_mask:
                mask_sb = persist.tile([1, T], F32R, tag="mask_sb", name="mask_sb")
                nc.gpsimd.dma_start(
                    out=mask_sb, in_=bass.AP(mask, 0, [[0, 1], [1, T]])
                )
                ones10k = cst([1, 128], F32, "ones10k_f")
                nc.vector.memset(ones10k, 10000.0)
                ones10k_r = cst([1, 128], F32R, "ones10k")
                nc.vector.tensor_copy(ones10k_r, ones10k)
                ones10k = ones10k_r

            # pin the ACT function table: sigmoid_and_others contains BOTH
            # Sigmoid and Tanh, so forcing Sigmoid first avoids a second
            # 1.3us table load when Tanh would otherwise pick its own table
            actpin = cst([128, 1], F32, "actpin")
            nc.scalar.activation(
                actpin, bias_x[0], mybir.ActivationFunctionType.Sigmoid
            )

            # persistent scan output, per o-half strip; bf16 so the output
            # transposes run in the 1 cyc/row PE mode (the scan state is
            # fp32 internally; only the stored H is rounded)
            H = [
                persist.tile([128, T], BF16, tag=f"H{oh}", name=f"H{oh}")
                for oh in range(2)
            ]

            NB = TC // 128  # t-blocks per chunk
            x_v = x_in[:, :].rearrange("(c n p) d -> c p n d", p=128, n=NB)
            out_v = out[:, :].rearrange("(q n p) o -> q p n o", p=128, n=NB)


            # ---- main pipeline --------------------------------------
            for c in range(N_CHUNKS):
                t0 = c * TC
                nat = nat_pool.tile([128, NB, D], F32R, tag="nat", name=f"nat{c}")
                nc.sync.dma_start(out=nat, in_=x_v[c])

                # input transpose: [128t, 128d] blocks -> psT [128d, 2*TC]
                # (kh halves side by side), one PSUM->SBUF copy per chunk
                pt = ps_in.tile([128, 2 * TC], F32R, tag="psT")
                for kh in range(2):
                    for n in range(NB):
                        nc.tensor.transpose(
                            pt[:, kh * TC + n * 128 : kh * TC + (n + 1) * 128],
                            nat[:, n, kh * 128 : (kh + 1) * 128],
                            ident_r,
                        )
                rhs = rhs_pool.tile([128, 2 * TC], F32R, tag="rs", name=f"rs{c}")
                if c in W1_DVE:
                    nc.vector.tensor_copy(rhs, pt)
                else:
                    nc.scalar.copy(rhs, pt)

                # gates for both o-halves live side by side on the free
                # axis: [:, 0:TC] is oh=0, [:, TC:2TC] is oh=1.
                # xn = tanh(-z_x - b_in) = -x, ag = sigmoid(-z_f - b_f)
                # = 1 - f: both negations are free via the ACT scale input
                xn = gate_pool.tile([128, 2 * TC], BF16, tag="xn")
                ag = gate_pool.tile([128, 2 * TC], BF16, tag="ag")
                for oh in range(2):
                    sl = slice(oh * TC, (oh + 1) * TC)
                    # z_f -> 1-sigmoid (emitted first: the first activation
                    # executed being Sigmoid pins the sigmoid_and_others
                    # table, which also serves Tanh -> one table load)
                    zf = ps_z.tile([128, TC], F32, tag="z")
                    n_acc = 3 if with_mask else 2
                    for kh in range(2):
                        nc.tensor.matmul(
                            zf,
                            wT[1][kh][:, oh * 128 : (oh + 1) * 128],
                            rhs[:, kh * TC : (kh + 1) * TC],
                            start=(kh == 0),
                            stop=(kh == n_acc - 1),
                        )
                    if with_mask:
                        nc.tensor.matmul(
                            zf,
                            ones10k,
                            mask_sb[:, t0 : t0 + TC],
                            start=False,
                            stop=True,
                        )
                    nc.scalar.activation(
                        ag[:, sl], zf, mybir.ActivationFunctionType.Sigmoid,
                        bias=bias_f[oh], scale=-1.0,
                    )

                    # z_x -> -tanh
                    z = ps_z.tile([128, TC], F32, tag="z")
                    for kh in range(2):
                        nc.tensor.matmul(
                            z,
                            wT[0][kh][:, oh * 128 : (oh + 1) * 128],
                            rhs[:, kh * TC : (kh + 1) * TC],
                            start=(kh == 0),
                            stop=(kh == 1),
                        )
                    nc.scalar.activation(
                        xn[:, sl], z, mybir.ActivationFunctionType.Tanh,
                        bias=bias_x[oh], scale=-1.0,
                    )

                # b = f*x = (ag - 1) * (-x): tensor_scalar runs in 4x bf16
                # mode and tensor_tensor in 2x, beating the fused 1x op
                am = gate_pool.tile([128, 2 * TC], BF16, tag="am")
                nc.vector.tensor_scalar(
                    am, ag, 1.0, 0.0,
                    op0=mybir.AluOpType.subtract,
                    op1=mybir.AluOpType.add,
                )
                bn = gate_pool.tile([128, 2 * TC], BF16, tag="bn")
                nc.vector.tensor_tensor(bn, am, xn, op=mybir.AluOpType.mult)

                # h_t = a*h_{t-1} + b  (fp32 internal state per scan instr)
                for oh in range(2):
                    sl = slice(oh * TC, (oh + 1) * TC)
                    init = 0.0 if c == 0 else H[oh][:, t0 - 1 : t0]
                    nc.vector.tensor_tensor_scan(
                        H[oh][:, t0 : t0 + TC],
                        ag[:, sl],
                        bn[:, sl],
                        init,
                        op0=mybir.AluOpType.mult,
                        op1=mybir.AluOpType.add,
                    )

                # output transpose (bf16, 1 cyc/row): po[t, n*256+o] rows
                po = ps_out.tile([128, NB * 256], BF16)
                for n in range(NB):
                    tb = t0 + n * 128
                    for oh in range(2):
                        nc.tensor.transpose(
                            po[:, n * 256 + oh * 128 : n * 256 + oh * 128 + 128],
                            H[oh][:, tb : tb + 128],
                            ident_h,
                        )
                # stage bf16 PSUM -> SBUF (DVE 2x mode), then flush with a
                # casting SWDGE DMA (bf16 -> fp32, exact widening)
                onat = onat_pool.tile([128, NB, 256], BF16, tag="onat",
                                      name=f"onat{c}")
                nc.vector.tensor_copy(
                    onat.rearrange("p n o -> p (n o)"), po
                )
                nc.gpsimd.dma_start(out=out_v[c], in_=onat)

    nc.compile()
    return nc


def _get_module(with_mask: bool):
    key = bool(with_mask)
    if key not in _cache:
        _cache[key] = build_module(key)
    return _cache[key]


def kernel(**inputs):
    inp = np.ascontiguousarray(np.asarray(inputs["inputs"], dtype=np.float32))
    msk = np.ascontiguousarray(np.asarray(inputs["mask"], dtype=np.float32))
    w_in = np.ascontiguousarray(np.asarray(inputs["W_in"], dtype=np.float32))
    b_in = np.ascontiguousarray(np.asarray(inputs["b_in"], dtype=np.float32))
    w_f = np.ascontiguousarray(np.asarray(inputs["W_f"], dtype=np.float32))
    b_f = np.ascontiguousarray(np.asarray(inputs["b_f"], dtype=np.float32))

    with_mask = bool(np.any(msk != 0.0))
    nc = _get_module(with_mask)

    in_maps = []
    for c in range(N_CORES):
        m = {
            "x": inp[c],
            "w_in": w_in,
            "b_in": b_in,
            "w_f": w_f,
            "b_f": b_f,
        }
        if with_mask:
            m["mask"] = msk[c]
        in_maps.append(m)

    res = run_bass_kernel_spmd(nc, in_maps, core_ids=list(range(N_CORES)))
    return np.stack([res.results[c]["out"] for c in range(N_CORES)], axis=0)


# revision 16
# speedup vs baseline: 1.0977x; 1.0420x over previous
"""QRNN forget-mult kernel for Trainium2 (Bass/Tile), 8-core batch-parallel.

Reference computation (per batch b):
    x = tanh(inputs @ W_in.T + b_in)            # (T, D)
    f = sigmoid(inputs @ W_f.T + b_f + 10000*mask)
    h_t = f_t*x_t + (1-f_t)*h_{t-1},  h_{-1} = 0

Shapes: B=8, T=4096, D_IN=D_OUT=256, fp32.

Sharding: batch across the 8 NeuronCores (core c <- batch c). The
recurrence is independent per (batch, feature) so no communication.

Per-core dataflow, with engine work balanced so everything sits at or
under the ~24.5us DMA roofline (4MB in + 4MB out + weights @ 360GB/s):
  DMA in    : inputs[c] natural [128t, d] chunks            (SP HWDGE)
  PE        : transpose input tiles -> psT [128d, t] (fp32r, 1.5cyc/row)
  DVE/ACT   : copy psT PSUM->SBUF rhs (split across chunks to balance)
  PE        : z_x, z_f = W^T.T @ rhs accumulated over d (fp32r 1cyc/row)
  ACT       : xg = tanh(z_x + b_in); fg = sigmoid(z_f + b_f)  -> bf16
  DVE       : ag = 1 - fg          (tensor_scalar, 4x bf16 mode)
  Pool      : bn = fg * xg         (tensor_tensor)
  DVE       : H = scan(ag, bn): H_t = ag_t*H_{t-1} + bn_t (fp32 state,
              bf16 stored output)
  PE        : transpose H (bf16 -> 1 cyc/row) -> PSUM bf16
  DVE       : copy PSUM->SBUF bf16 (2x mode)
  Pool SWDGE: casting DMA bf16 -> fp32 natural [t, o] rows (exact widen)
"""

import os
import sys

import numpy as np

for _p in ("/opt/trn_rl_repo",):
    if _p not in sys.path and os.path.isdir(_p):
        sys.path.insert(0, _p)

import concourse.bacc as bacc
import concourse.bass as bass
import concourse.mybir as mybir
import concourse.tile as tile
from concourse.bass_utils import run_bass_kernel_spmd
from concourse.masks import make_identity

B, T, D = 8, 4096, 256
N_CORES = 8
TC = 512          # time-chunk per pipeline iteration
N_CHUNKS = T // TC
F32 = mybir.dt.float32
F32R = mybir.dt.float32r
BF16 = mybir.dt.bfloat16

# chunks whose psT->rhs copy runs on DVE (rest on ACT); tuned for balance
W1_DVE = {0, 3, 6}

_cache = {}


def build_module(with_mask: bool):
    nc = bacc.Bacc("TRN2")

    # x and the weight matrices are declared float32r (same 4-byte layout,
    # np.float32 on the host): their transposes then run in the faster
    # 1.5 cyc/row fp32r PE mode and satisfy the fp32r producer-rounding rule
    x_in = nc.dram_tensor("x", [T, D], F32R, kind="ExternalInput")
    w_in = nc.dram_tensor("w_in", [D, D], F32R, kind="ExternalInput")
    b_in = nc.dram_tensor("b_in", [D], F32, kind="ExternalInput")
    w_f = nc.dram_tensor("w_f", [D, D], F32R, kind="ExternalInput")
    b_f = nc.dram_tensor("b_f", [D], F32, kind="ExternalInput")
    mask = None
    if with_mask:
        mask = nc.dram_tensor("mask", [T, 1], F32, kind="ExternalInput")
    out = nc.dram_tensor("out", [T, D], F32, kind="ExternalOutput")

    with tile.TileContext(nc) as tc:
        with (
            tc.tile_pool(name="consts", bufs=1) as consts,
            tc.tile_pool(name="persist", bufs=1) as persist,
            tc.tile_pool(name="nat", bufs=6) as nat_pool,
            tc.tile_pool(name="rhs", bufs=3) as rhs_pool,
            tc.tile_pool(name="gates", bufs=3) as gate_pool,
            tc.tile_pool(name="onat", bufs=3) as onat_pool,
            tc.tile_pool(name="ps_in", bufs=2, space="PSUM") as ps_in,
            tc.tile_pool(name="ps_z", bufs=2, space="PSUM") as ps_z,
            tc.tile_pool(name="ps_out", bufs=2, space="PSUM") as ps_out,
        ):
            # ---- one-time setup -------------------------------------
            def cst(shape, dtype, nm):
                return consts.tile(shape, dtype, name=nm, tag=nm)

            # identity for fp32 transposes, an fp32r-rounded copy for the
            # fp32r input transposes (verifier: fp32r matmul operands must
            # come from an fp32r-rounding producer), and a bf16 copy for
            # the 1 cyc/row output transposes
            ident = cst([128, 128], F32, "ident")
            make_identity(nc, ident)
            ident_r = cst([128, 128], F32R, "ident_r")
            nc.vector.tensor_copy(ident_r, ident)
            ident_h = cst([128, 128], BF16, "ident_h")
            nc.vector.tensor_copy(ident_h, ident)

            # negated biases [128, 1] per o-half: the activations compute
            # ag = sigmoid(-z_f - b_f) = 1 - f and xn = tanh(-z_x - b_in)
            # = -x via scale=-1, so the gate math needs -b
            bias_x = []
            bias_f = []
            for oh in range(2):
                bx = cst([128, 1], F32, f"bx{oh}")
                nc.scalar.dma_start(
                    out=bx, in_=bass.AP(b_in, oh * 128, [[1, 128], [0, 1]])
                )
                bxn = cst([128, 1], F32, f"bxn{oh}")
                nc.vector.tensor_scalar(
                    bxn, bx, -1.0, 0.0,
                    op0=mybir.AluOpType.mult, op1=mybir.AluOpType.add,
                )
                bf = cst([128, 1], F32, f"bf{oh}")
                nc.scalar.dma_start(
                    out=bf, in_=bass.AP(b_f, oh * 128, [[1, 128], [0, 1]])
                )
                bfn = cst([128, 1], F32, f"bfn{oh}")
                nc.vector.tensor_scalar(
                    bfn, bf, -1.0, 0.0,
                    op0=mybir.AluOpType.mult, op1=mybir.AluOpType.add,
                )
                bias_x.append(bxn)
                bias_f.append(bfn)

            # weights: load natural [128o, 256d], PE-transpose to
            # wT[gate][kh] = [128d, 256o]
            wT = [[None, None], [None, None]]
            for g, w_dram in enumerate((w_in, w_f)):
                wnat = []
                for oh in range(2):
                    wn = cst([128, D], F32R, f"wnat{g}{oh}")
                    nc.scalar.dma_start(
                        out=wn, in_=w_dram[oh * 128 : (oh + 1) * 128, :]
                    )
                    wnat.append(wn)
                for kh in range(2):
                    pw = ps_z.tile([128, TC], F32R, tag="z", name=f"pw{g}{kh}")
                    for oh in range(2):
                        nc.tensor.transpose(
                            pw[:, oh * 128 : (oh + 1) * 128],
                            wnat[oh][:, kh * 128 : (kh + 1) * 128],
                            ident_r,
                        )
                    wt = cst([128, D], F32R, f"wT{g}{kh}")
                    nc.vector.tensor_copy(wt, pw[:, 0:D])

                    wT[g][kh] = wt

            mask_sb = None
            ones10k = None
            if with_mask:
                mask_sb = persist.tile([1, T], F32R, tag="mask_sb", name="mask_sb")
                nc.gpsimd.dma_start(
                    out=mask_sb, in_=bass.AP(mask, 0, [[0, 1], [1, T]])
                )
                ones10k = cst([1, 128], F32, "ones10k_f")
                nc.vector.memset(ones10k, 10000.0)
                ones10k_r = cst([1, 128], F32R, "ones10k")
                nc.vector.tensor_copy(ones10k_r, ones10k)
                ones10k = ones10k_r

            # pin the ACT function table: sigmoid_and_others contains BOTH
            # Sigmoid and Tanh, so forcing Sigmoid first avoids a second
            # 1.3us table load when Tanh would otherwise pick its own table
            actpin = cst([128, 1], F32, "actpin")
            nc.scalar.activation(
                actpin, bias_x[0], mybir.ActivationFunctionType.Sigmoid
            )

            # persistent scan output, per o-half strip; bf16 so the output
            # transposes run in the 1 cyc/row PE mode (the scan state is
            # fp32 internally; only the stored H is rounded)
            H = [
                persist.tile([128, T], BF16, tag=f"H{oh}", name=f"H{oh}")
                for oh in range(2)
            ]

            NB = TC // 128  # t-blocks per chunk
            x_v = x_in[:, :].rearrange("(c n p) d -> c p n d", p=128, n=NB)
            out_v = out[:, :].rearrange("(q n p) o -> q p n o", p=128, n=NB)


            # ---- main pipeline --------------------------------------
            for c in range(N_CHUNKS):
                t0 = c * TC
                nat = nat_pool.tile([128, NB, D], F32R, tag="nat", name=f"nat{c}")
                nc.sync.dma_start(out=nat, in_=x_v[c])

                # input transpose: [128t, 128d] blocks -> psT [128d, 2*TC]
                # (kh halves side by side), one PSUM->SBUF copy per chunk
                pt = ps_in.tile([128, 2 * TC], F32R, tag="psT")
                for kh in range(2):
                    for n in range(NB):
                        nc.tensor.transpose(
                            pt[:, kh * TC + n * 128 : kh * TC + (n + 1) * 128],
                            nat[:, n, kh * 128 : (kh + 1) * 128],
                            ident_r,
                        )
                rhs = rhs_pool.tile([128, 2 * TC], F32R, tag="rs", name=f"rs{c}")
                if c in W1_DVE:
                    nc.vector.tensor_copy(rhs, pt)
                else:
                    nc.scalar.copy(rhs, pt)

                # gates for both o-halves live side by side on the free
                # axis: [:, 0:TC] is oh=0, [:, TC:2TC] is oh=1.
                # xn = tanh(-z_x - b_in) = -x, ag = sigmoid(-z_f - b_f)
                # = 1 - f: both negations are free via the ACT scale input
                xn = gate_pool.tile([128, 2 * TC], BF16, tag="xn")
                ag = gate_pool.tile([128, 2 * TC], BF16, tag="ag")
                for oh in range(2):
                    sl = slice(oh * TC, (oh + 1) * TC)
                    # z_f -> 1-sigmoid (emitted first: the first activation
                    # executed being Sigmoid pins the sigmoid_and_others
                    # table, which also serves Tanh -> one table load)
                    zf = ps_z.tile([128, TC], F32, tag="z")
                    n_acc = 3 if with_mask else 2
                    for kh in range(2):
                        nc.tensor.matmul(
                            zf,
                            wT[1][kh][:, oh * 128 : (oh + 1) * 128],
                            rhs[:, kh * TC : (kh + 1) * TC],
                            start=(kh == 0),
                            stop=(kh == n_acc - 1),
                        )
                    if with_mask:
                        nc.tensor.matmul(
                            zf,
                            ones10k,
                            mask_sb[:, t0 : t0 + TC],
                            start=False,
                            stop=True,
                        )
                    nc.scalar.activation(
                        ag[:, sl], zf, mybir.ActivationFunctionType.Sigmoid,
                        bias=bias_f[oh], scale=-1.0,
                    )

                    # z_x -> -tanh
                    z = ps_z.tile([128, TC], F32, tag="z")
                    for kh in range(2):
                        nc.tensor.matmul(
                            z,
                            wT[0][kh][:, oh * 128 : (oh + 1) * 128],
                            rhs[:, kh * TC : (kh + 1) * TC],
                            start=(kh == 0),
                            stop=(kh == 1),
                        )
                    nc.scalar.activation(
                        xn[:, sl], z, mybir.ActivationFunctionType.Tanh,
                        bias=bias_x[oh], scale=-1.0,
                    )

                # b = f*x = (ag - 1) * (-x): tensor_scalar runs in 4x bf16
                # mode and tensor_tensor in 2x, beating the fused 1x op
                am = gate_pool.tile([128, 2 * TC], BF16, tag="am")
                nc.vector.tensor_scalar(
                    am, ag, 1.0, 0.0,
                    op0=mybir.AluOpType.subtract,
                    op1=mybir.AluOpType.add,
                )
                bn = gate_pool.tile([128, 2 * TC], BF16, tag="bn")
                nc.vector.tensor_tensor(bn, am, xn, op=mybir.AluOpType.mult)

                # h_t = a*h_{t-1} + b  (fp32 internal state per scan instr)
                for oh in range(2):
                    sl = slice(oh * TC, (oh + 1) * TC)
                    init = 0.0 if c == 0 else H[oh][:, t0 - 1 : t0]
                    nc.vector.tensor_tensor_scan(
                        H[oh][:, t0 : t0 + TC],
                        ag[:, sl],
                        bn[:, sl],
                        init,
                        op0=mybir.AluOpType.mult,
                        op1=mybir.AluOpType.add,
                    )

                # output transpose (bf16, 1 cyc/row): po[t, n*256+o] rows
                po = ps_out.tile([128, NB * 256], BF16)
                for n in range(NB):
                    tb = t0 + n * 128
                    for oh in range(2):
                        nc.tensor.transpose(
                            po[:, n * 256 + oh * 128 : n * 256 + oh * 128 + 128],
                            H[oh][:, tb : tb + 128],
                            ident_h,
                        )
                # stage bf16 PSUM -> SBUF (DVE 2x mode), then flush with a
                # casting SWDGE DMA (bf16 -> fp32, exact widening)
                onat = onat_pool.tile([128, NB, 256], BF16, tag="onat",
                                      name=f"onat{c}")
                nc.vector.tensor_copy(
                    onat.rearrange("p n o -> p (n o)"), po
                )
                nc.gpsimd.dma_start(out=out_v[c], in_=onat)

    nc.compile()
    return nc


def _get_module(with_mask: bool):
    key = bool(with_mask)
    if key not in _cache:
        _cache[key] = build_module(key)
    return _cache[key]


def kernel(**inputs):
    inp = np.ascontiguousarray(np.asarray(inputs["inputs"], dtype=np.float32))
    msk = np.ascontiguousarray(np.asarray(inputs["mask"], dtype=np.float32))
    w_in = np.ascontiguousarray(np.asarray(inputs["W_in"], dtype=np.float32))
    b_in = np.ascontiguousarray(np.asarray(inputs["b_in"], dtype=np.float32))
    w_f = np.ascontiguousarray(np.asarray(inputs["W_f"], dtype=np.float32))
    b_f = np.ascontiguousarray(np.asarray(inputs["b_f"], dtype=np.float32))

    with_mask = bool(np.any(msk != 0.0))
    nc = _get_module(with_mask)

    in_maps = []
    for c in range(N_CORES):
        m = {
            "x": inp[c],
            "w_in": w_in,
            "b_in": b_in,
            "w_f": w_f,
            "b_f": b_f,
        }
        if with_mask:
            m["mask"] = msk[c]
        in_maps.append(m)

    res = run_bass_kernel_spmd(nc, in_maps, core_ids=list(range(N_CORES)))
    return np.stack([res.results[c]["out"] for c in range(N_CORES)], axis=0)
